# revision 1
# baseline (speedup 1.0000x reference)
"""Per-sample dynamic 3x3 conv (DCConv2d) on 8 Trainium2 NeuronCores.

Strategy: pure data parallel, B_LOCAL=16 samples/core, processed in 4 groups
of 4 samples packed as block-diagonal 32x32 tiles on the 128x128 PE array.
The 3x3 conv is 9 PSUM-accumulated bf16 matmuls (one per tap) over host-
padded images resident in SBUF as [128 partitions = (sample, channel), 130,
130] bf16. Per-sample weights (inputs_se @ bank) are generated on-device in
f32 (one ACT scaled-copy + 7 fused multiply-adds), then cast into the bf16
block-diagonal stationary tile.

Engine/queue plan (the two HWDGE rings never mix loads with stores):
  sync ring    : small one-time loads, then x-image loads for groups 0/1
                 (group 0 row-split so chunk-0 matmuls can start early)
  scalar ring  : ALL output stores (even chunks drained by ACT, odd by DVE)
  gpsimd SWDGE : x-image loads for groups 2/3 + weight-gen for groups 1-3
  DVE          : weight-gen for group 0 (critical path) + odd-chunk drains
  PE           : warmup matmuls (p-state ramp) then 1152 conv matmuls,
                 emitted chunk-pair/tap-major so each LDWEIGHTS covers two
                 512-column streams.
Output is stored chunk-contiguous ([g, chunk, 128, 512] f32) and unpermuted
on host; x is pre-padded/packed on host so each group is one contiguous
[128 x 33800B] bf16 DMA.
"""

import numpy as np
import ml_dtypes

import concourse.bass as bass
import concourse.mybir as mybir
import concourse.tile as tile
from concourse.bass_utils import run_bass_kernel_spmd

N_CORES = 8
B, C, H, W = 128, 32, 128, 128
O = 32
NUM = 8
KK = 3
B_LOCAL = B // N_CORES          # 16
GROUP = 4                       # samples packed per PE pass
N_GROUPS = B_LOCAL // GROUP     # 4
HP, WP = H + 2, W + 2           # zero-padded image dims
ROWS_PER_CHUNK = 4              # output rows per matmul chunk (N = 4*128 = 512)
N_CHUNKS = H // ROWS_PER_CHUNK  # 32
NTAPS = KK * KK                 # 9
WARMUP_MM = 72                  # PE warmup matmuls (N=256 each)

F32 = mybir.dt.float32
BF16 = mybir.dt.bfloat16
BF16_NP = ml_dtypes.bfloat16


def _split_multiwait_insts(nc):
    """This walrus build encodes at most one sync-wait per instruction; Tile's
    tail drain carries one wait per hardware proc used. Split the extras into
    single-wait NOPs on the same engine, inserted just before."""
    for f in nc.m.functions:
        for blk in f.blocks:
            new_list = []
            changed = False
            for inst in blk.instructions:
                si = inst.sync_info
                if si is not None and len(si.on_wait) > 1:
                    waits = list(si.on_wait)
                    for j, w in enumerate(waits[:-1]):
                        new_list.append(
                            mybir.InstNoOp(
                                name=f"{inst.name}-ws-{j}",
                                engine=inst.engine,
                                ins=[],
                                outs=[],
                                sync_info=mybir.SyncInfo(on_wait=[w], on_update=[]),
                            )
                        )
                    inst.sync_info = mybir.SyncInfo(
                        on_wait=[waits[-1]], on_update=list(si.on_update)
                    )
                    changed = True
                new_list.append(inst)
            if changed:
                blk.instructions = new_list


def build_program(b_local=B_LOCAL, split_waits=True, reps=1):
    n_groups = b_local // GROUP
    nc = bass.Bass(
        "TRN2",
        target_bir_lowering=False,
        debug=False,
        num_devices=N_CORES,
        enable_partition_id=False,
    )
    # x: host-padded bf16, x[g, 32*s+i, r, c] = xpad[g*4+s, i, r, c]
    x_d = nc.dram_tensor("x", [n_groups, 128, HP, WP], BF16, kind="ExternalInput").ap()
    # wb: host-permuted bf16 weight bank replicated onto all four partition
    # groups, wb[32s+i, n, t, o] = weight[o*288+i*9+t, n]
    wb_d = nc.dram_tensor("wb", [128, NUM, NTAPS, O], BF16, kind="ExternalInput").ap()
    # se: host-replicated, se[32*s+i, g, n] = inputs_se[core_base + g*4+s, n]
    se_d = nc.dram_tensor("se", [128, n_groups, NUM], F32, kind="ExternalInput").ap()
    # bias replicated 4x across partition groups: [128, 1]
    bias_d = nc.dram_tensor("bias", [128, 1], F32, kind="ExternalInput").ap()
    # y: partition-major, y[g, 32*s+o, ci, 128*r+w] so a chunk PAIR is one
    # 4KB-per-partition contiguous store; host unpermutes
    y_d = nc.dram_tensor(
        "y", [n_groups, 128, N_CHUNKS, ROWS_PER_CHUNK * W], F32, kind="ExternalOutput"
    ).ap()

    with tile.TileContext(nc) as tc:
        with (
            tc.tile_pool(name="xpool", bufs=1) as xpool,
            tc.tile_pool(name="wbpool", bufs=1) as wbpool,
            tc.tile_pool(name="wbdpool", bufs=1) as wbdpool,
            tc.tile_pool(name="wfin", bufs=1) as wfin_pool,
            tc.tile_pool(name="sepool", bufs=1) as sepool,
            tc.tile_pool(name="warm", bufs=1) as warmpool,
            tc.tile_pool(name="outp", bufs=6) as outp,
            tc.tile_pool(name="psum", bufs=8, space="PSUM") as psump,
        ):
            # --- ACT table preload + one-time loads + zero-fills ----------
            scratch = warmpool.tile([128, 1], F32, tag="scratch")
            nc.gpsimd.memset(scratch[:], 0.0)
            nc.scalar.activation(
                scratch[:], scratch[:], mybir.ActivationFunctionType.Identity
            )
            wbsb = wbpool.tile([128, NUM, NTAPS, O], BF16)
            nc.sync.dma_start(out=wbsb[:], in_=wb_d[:])
            se_sb = sepool.tile([128, n_groups, NUM], F32)
            nc.scalar.dma_start(out=se_sb[:], in_=se_d[:])
            bias_sb = sepool.tile([128, 1], F32)
            nc.scalar.dma_start(out=bias_sb[:], in_=bias_d[:])
            # warmup operands + block-diagonal stationary bank (off-diagonal
            # zeros persist across groups and reps)
            warm = warmpool.tile([128, 512], BF16, tag="warm")
            nc.gpsimd.memset(warm[:], 0.0)
            wbd = wbdpool.tile([128, n_groups, NTAPS, 128], BF16)
            nc.gpsimd.memset(wbd[:], 0.0)

            # weight-gen for groups 0/1 primes the first loop iteration; the
            # body refreshes them once their conv has finished, so later
            # iterations start with valid stationary weights.
            for g in range(min(2, n_groups)):
                for op in _weight_gen_ops(nc, g, wbd, wbsb, se_sb, wfin_pool):
                    op()

            # four persistent image tiles; groups 0/1 primed here (group 0
            # row-split so the first chunks' matmuls can start early), the
            # body reloads them for the next iteration once their conv is
            # done — the post-barrier critical path carries no DMA at all.
            xts = []
            for i in range(n_groups):
                xt_i = xpool.tile([128, HP, WP], BF16, tag=f"xt{i}")
                xts.append(xt_i)
            _load_rows(nc, xts[0], x_d, 0, first_small=True)
            if n_groups > 1:
                _load_rows(nc, xts[1], x_d, 1)

            # --- PE warmup: ramp the p-state while DMAs/weight-gen run ----
            wps = psump.tile([128, ROWS_PER_CHUNK * W], F32, tag="ps")
            for _ in range(WARMUP_MM):
                nc.tensor.matmul(wps[:, 0:256], warm[:, 0:128], warm[:, 256:512],
                                 start=True, stop=True)

            from contextlib import nullcontext

            # Four bodies per hardware-loop iteration: the state rotation
            # (weights/images for groups 0/1 refreshed once their readers
            # finish) lets consecutive bodies stream back-to-back with no
            # gap, so only every fourth body pays the For_i barrier.
            if reps > 1:
                assert reps % 4 == 0, "reps must be divisible by 4"
                rep_loop, n_bodies = tc.For_i(0, reps // 4, 1), 4
            else:
                rep_loop, n_bodies = nullcontext(), 1
            with rep_loop:
                for b in range(n_bodies):
                    _emit_body(nc, tc, n_groups, x_d, y_d, wbd, wbsb, se_sb,
                               bias_sb, xts, wfin_pool, outp, psump,
                               reload_head=(reps > 1),
                               tail_warm=(warm if reps > 1 and
                                          b == n_bodies - 1 else None))

    if split_waits:
        _split_multiwait_insts(nc)
    return nc


def _weight_gen_ops(nc, g, wbd, wbsb, se_sb, wfin_pool):
    """Closures computing wfin[(s,i), t, o] = sum_n se[(s,i),g,n] *
    wbsb[(s,i),n,t,o] in bf16 (ACT scaled copy + 7 fused DVE multiply-adds),
    then copying the four diagonal blocks into the stationary bank.
    Returned one-per-instruction so the caller can stagger the emission
    through a conv chunk loop without stalling the DVE epilogue queue."""
    mult = mybir.AluOpType.mult
    add = mybir.AluOpType.add
    ident = mybir.ActivationFunctionType.Identity
    wfin = wfin_pool.tile([128, NTAPS, O], BF16, tag="wfin")
    ops = [
        lambda: nc.scalar.activation(
            wfin[:], wbsb[:, 0], ident, scale=se_sb[:, g, 0:1]
        )
    ]
    for n in range(1, NUM):
        ops.append(
            lambda n=n: nc.vector.scalar_tensor_tensor(
                wfin[:], wbsb[:, n], se_sb[:, g, n : n + 1], wfin[:], mult, add
            )
        )
    for s in range(GROUP):
        ops.append(
            lambda s=s: nc.vector.tensor_copy(
                wbd[32 * s : 32 * (s + 1), g, :, 32 * s : 32 * (s + 1)],
                wfin[32 * s : 32 * (s + 1)],
            )
        )
    return ops


def _load_rows(nc, t, x_d, g, first_small=False):
    """x-image load as 8 row-chunk DMAs. The split matters: SDMA engines
    round-robin between queues at packet granularity, so one monolithic
    load (33.8KB descriptors) would starve the store ring to ~12% of
    bandwidth while it runs."""
    bounds = (0, 18, 34, 50, 66, 82, 98, 114, HP) if first_small else (
        0, 17, 34, 50, 66, 82, 98, 114, HP
    )
    for r0, r1 in zip(bounds, bounds[1:]):
        nc.sync.dma_start(out=t[:, r0:r1, :], in_=x_d[g, :, r0:r1, :])


def _emit_body(nc, tc, n_groups, x_d, y_d, wbd, wbsb, se_sb, bias_sb,
               xts, wfin_pool, outp, psump, reload_head=False,
               tail_warm=None):
    ident = mybir.ActivationFunctionType.Identity

    # Rotation (n_groups == 4): groups 0/1's weights and images are valid
    # when the body starts (pre-loop priming, or the previous iteration's
    # refreshes). This body loads images for groups 2/3 during groups 0/1's
    # conv, generates groups 2/3's weights during group 1's conv (group 0's
    # conv keeps a clean DVE for its PSUM drains right after the barrier),
    # and refreshes groups 0/1's weights + images once their readers finish.
    gen_during = {1: (2, 3), 2: (0,), 3: (1,)} if n_groups == 4 else {
        g: (g + 1,) for g in range(n_groups - 1)
    }
    load_during = {0: 2, 1: 3}
    if reload_head:
        load_during.update({2: 0, 3: 1})

    # --- conv: 32 chunks of 4 output rows per group, chunk-pair/tap-major
    for g in range(n_groups):
        lg = load_during.get(g)
        if lg is not None and lg < n_groups:
            _load_rows(nc, xts[lg], x_d, lg)
        pending = []
        for gg in gen_during.get(g, ()):
            if gg < n_groups:
                pending += _weight_gen_ops(nc, gg, wbd, wbsb, se_sb, wfin_pool)
        xt = xts[g]
        for cp in range(N_CHUNKS // 2):
            pss = []
            for ci in (2 * cp, 2 * cp + 1):
                ps = psump.tile([128, ROWS_PER_CHUNK * W], F32, tag="ps")
                pss.append((ci, ps))
            for tap in range(NTAPS):
                kh, kw = divmod(tap, KK)
                for ci, ps in pss:
                    h0 = ci * ROWS_PER_CHUNK
                    nc.tensor.matmul(
                        ps[:],
                        wbd[:, g, tap, :],
                        xt[:, h0 + kh : h0 + kh + ROWS_PER_CHUNK, kw : kw + W],
                        start=(tap == 0),
                        stop=(tap == NTAPS - 1),
                    )
            # pair drain into one double tile; one 4KB-per-partition store
            # per pair from the scalar engine (never sharing a ring with the
            # x loads). Group 0 drains entirely on ACT — right after the
            # barrier the scheduler front-loads the weight-gen chains on
            # DVE, and ACT is the engine with room.
            ob = outp.tile([128, 2, ROWS_PER_CHUNK * W], F32)
            for k, (ci, ps) in enumerate(pss):
                if (g == 0 and cp < 11) or k == 0:
                    nc.scalar.activation(
                        ob[:, k], ps[:], ident, bias=bias_sb[:, 0:1]
                    )
                else:
                    nc.vector.tensor_scalar_add(ob[:, k], ps[:], bias_sb[:, 0:1])
            nc.scalar.dma_start(
                out=y_d[g, :, 2 * cp : 2 * cp + 2, :], in_=ob[:]
            )
            if cp >= 1:
                for op in pending[: 2]:
                    op()
                del pending[: 2]
        for op in pending:
            op()

    if tail_warm is not None:
        # keep the PE clock ramped through the loop barrier's flush window
        wps = psump.tile([128, ROWS_PER_CHUNK * W], F32, tag="ps")
        for _ in range(12):
            nc.tensor.matmul(wps[:, 0:256], tail_warm[:, 0:128],
                             tail_warm[:, 256:512], start=True, stop=True)


def _host_prep(inputs, inputs_se, weight, bias):
    """Shard + relayout the inputs for the 8 per-core programs."""
    inputs = np.asarray(inputs, dtype=np.float32)
    inputs_se = np.asarray(inputs_se, dtype=np.float32)
    weight = np.asarray(weight, dtype=np.float32)
    bias = np.asarray(bias, dtype=np.float32)

    # padded bf16 images: xpad[b, i, 1:129, 1:129] = x[b, i]
    xpad = np.zeros((B, C, HP, WP), dtype=BF16_NP)
    xpad[:, :, 1 : H + 1, 1 : W + 1] = inputs.astype(BF16_NP)

    # wb[32s+i, n, t, o] = weight[o*288 + i*9 + t, n], replicated 4x
    wb = weight.reshape(O, C, NTAPS, NUM).transpose(1, 3, 2, 0)
    wbh = np.tile(
        np.ascontiguousarray(wb).astype(BF16_NP), (GROUP, 1, 1, 1)
    )  # [128, 8, 9, 32]
    bias_rep = np.ascontiguousarray(
        np.tile(bias, GROUP)[:, None].astype(np.float32)
    )  # [128, 1]

    in_maps = []
    for core in range(N_CORES):
        b0 = core * B_LOCAL
        # x[g, 32s+i, r, c] with b = b0 + g*4 + s
        xh = xpad[b0 : b0 + B_LOCAL].reshape(N_GROUPS, 128, HP, WP)
        se_loc = inputs_se[b0 : b0 + B_LOCAL]           # [16, 8]
        # se[32*s+i, g, n] = se_loc[g*4+s, n]
        se_exp = np.repeat(
            se_loc.reshape(N_GROUPS, GROUP, NUM).transpose(1, 0, 2), 32, axis=0
        )
        in_maps.append(
            {
                "x": np.ascontiguousarray(xh),
                "wb": wbh,
                "se": np.ascontiguousarray(se_exp, dtype=np.float32),
                "bias": bias_rep,
            }
        )
    return in_maps


def _host_unpack(per_core_y):
    """[n_cores][4, 128, 32, 512] f32 -> [B, O, H, W]."""
    y = np.stack(per_core_y)                       # [8, g, (s,o), ci, (r,w)]
    y = y.reshape(N_CORES, N_GROUPS, GROUP, O, N_CHUNKS, ROWS_PER_CHUNK, W)
    return np.ascontiguousarray(y.reshape(B, O, H, W))


_NC_CACHE = {}


def kernel(inputs, inputs_se, weight, bias):
    if "nc" not in _NC_CACHE:
        _NC_CACHE["nc"] = build_program()
    nc = _NC_CACHE["nc"]
    in_maps = _host_prep(inputs, inputs_se, weight, bias)
    res = run_bass_kernel_spmd(nc, in_maps, list(range(N_CORES)))
    return _host_unpack([res.results[i]["y"] for i in range(N_CORES)])



# revision 2
# speedup vs baseline: 1.2031x; 1.2031x over previous
"""Per-sample dynamic 3x3 conv (DCConv2d) on 8 Trainium2 NeuronCores — v3.

PE runs in 64x64 tiled mode (4 tiles): row half I = 2 samples' channels
block-diagonal, col half J = output chunk. Each N=512 matmul covers 2
samples x 1 chunk x 1 tap, and the 4 tiles stream concurrently, so the
per-instruction dispatch cost (~34-60ns) is amortized over 2 chunk-units —
measured ~2x the work-rate of the v1 full-array block-diagonal layout.
Output is stored bf16 (host converts to f32); x is host-column-padded
only, row edges handled by row-restricted matmuls.

Structure per core (16 samples = 4 groups of 4):
  x     [4, 128, 128, 130] bf16    (group, 32s+ch, row, col+1)
  wb    [128, 8, 9, 32]    bf16    replicated expert bank
  se    [128, 4, 8]        f32     replicated se coefficients
  bias  [128, 1]           f32     bias replicated on 32-partition blocks
  y     [4, 8, 2, 128, 2, 4, 128] bf16  (g, pass, ii, (jj,k,o), q, r, w)

Per group: 8 passes of 4 chunks x 4 samples. Tile (I=64*ii, J=64*jj)
accumulates chunks (4p+2*jj+q) for q=0,1 over 9 taps into PSUM bank
2q+ii+4*(p%2) partitions J..J+63 (4 banks per pass, ping-pong with the
other 4). Weights: per-group per-sample [9,32] bank (ACT+7 DVE gen) then
4 DVE copies into a persistent 2-sample block-diagonal [128,4g,9,64] bank.
Drains split ACT/DVE with fused bias + bf16 cast; stores on the scalar
HWDGE ring; x loads on the sync ring.
"""

import numpy as np
import ml_dtypes

import concourse.bass as bass
import concourse.mybir as mybir
import concourse.tile as tile
from concourse.bass_utils import run_bass_kernel_spmd

N_CORES = 8
B, C, H, W = 128, 32, 128, 128
O = 32
NUM = 8
KK = 3
B_LOCAL = B // N_CORES          # 16
GROUP = 4                       # samples per group
N_GROUPS = B_LOCAL // GROUP     # 4
WP = W + 2                      # column-padded row width
ROWS_PER_CHUNK = 4
N_CHUNKS = H // ROWS_PER_CHUNK  # 32
N_PASSES = N_CHUNKS // 4        # 8 passes/group, 4 chunks/pass
NTAPS = KK * KK
WARMUP_MM = 72

F32 = mybir.dt.float32
BF16 = mybir.dt.bfloat16
BF16_NP = ml_dtypes.bfloat16

# full-coverage tap (kh=1,kw=1) first so start=True writes every element
TAPS = [(1, 1), (0, 0), (0, 1), (0, 2), (1, 0), (1, 2), (2, 0), (2, 1), (2, 2)]


def _split_multiwait_insts(nc):
    """This walrus build encodes at most one sync-wait per instruction; Tile's
    tail drain carries one wait per hardware proc used. Split the extras into
    single-wait NOPs on the same engine, inserted just before."""
    for f in nc.m.functions:
        for blk in f.blocks:
            new_list = []
            changed = False
            for inst in blk.instructions:
                si = inst.sync_info
                if si is not None and len(si.on_wait) > 1:
                    waits = list(si.on_wait)
                    for j, w in enumerate(waits[:-1]):
                        new_list.append(
                            mybir.InstNoOp(
                                name=f"{inst.name}-ws-{j}",
                                engine=inst.engine,
                                ins=[],
                                outs=[],
                                sync_info=mybir.SyncInfo(on_wait=[w], on_update=[]),
                            )
                        )
                    inst.sync_info = mybir.SyncInfo(
                        on_wait=[waits[-1]], on_update=list(si.on_update)
                    )
                    changed = True
                new_list.append(inst)
            if changed:
                blk.instructions = new_list


def build_program(split_waits=True, reps=1):
    n_groups = N_GROUPS
    nc = bass.Bass(
        "TRN2",
        target_bir_lowering=False,
        debug=False,
        num_devices=N_CORES,
        enable_partition_id=False,
    )
    x_d = nc.dram_tensor("x", [n_groups, 128, H, WP], BF16,
                         kind="ExternalInput").ap()
    wb_d = nc.dram_tensor("wb", [128, NUM, NTAPS, O], BF16,
                          kind="ExternalInput").ap()
    se_d = nc.dram_tensor("se", [128, n_groups, NUM], F32,
                          kind="ExternalInput").ap()
    bias_d = nc.dram_tensor("bias", [128, 1], F32, kind="ExternalInput").ap()
    y_d = nc.dram_tensor(
        "y", [n_groups, N_PASSES, 2, 128, 2, ROWS_PER_CHUNK, W], BF16,
        kind="ExternalOutput"
    ).ap()

    with tile.TileContext(nc) as tc:
        with (
            tc.tile_pool(name="xpool", bufs=1) as xpool,
            tc.tile_pool(name="wbpool", bufs=1) as wbpool,
            tc.tile_pool(name="wfin", bufs=1) as wfin_pool,
            tc.tile_pool(name="sepool", bufs=1) as sepool,
            tc.tile_pool(name="warm", bufs=1) as warmpool,
            tc.tile_pool(name="outp", bufs=8) as outp,
            tc.tile_pool(name="psum", bufs=1, space="PSUM") as psump,
        ):
            # --- ACT table preload + one-time loads ------------------------
            scratch = warmpool.tile([128, 1], F32, tag="scratch")
            nc.gpsimd.memset(scratch[:], 0.0)
            nc.scalar.activation(
                scratch[:], scratch[:], mybir.ActivationFunctionType.Identity
            )
            wbsb = wbpool.tile([128, NUM, NTAPS, O], BF16)
            nc.sync.dma_start(out=wbsb[:], in_=wb_d[:])
            se_sb = sepool.tile([128, n_groups, NUM], F32)
            nc.scalar.dma_start(out=se_sb[:], in_=se_d[:])
            bias_sb = sepool.tile([128, 1], F32)
            nc.scalar.dma_start(out=bias_sb[:], in_=bias_d[:])
            warm = warmpool.tile([128, 576], BF16, tag="warm")
            nc.gpsimd.memset(warm[:], 0.0)

            # 8 persistent PSUM bank tiles, viewed [128, 4 rows, 128 cols]
            ps = [psump.tile([128, ROWS_PER_CHUNK, W], F32, tag=f"ps{b}",
                             name=f"ps{b}") for b in range(8)]

            # per-sample weights [128,(g),9,32] and the persistent 2-sample
            # block-diagonal stationary bank [128,(g),9,64] (off-diagonal
            # zeros persist; only diagonal blocks are rewritten per group)
            wfin_all = wfin_pool.tile([128, n_groups, NTAPS, O], BF16,
                                      tag="wfin", name="wfin_all")
            wbd2 = wfin_pool.tile([128, n_groups, NTAPS, 2 * O], BF16,
                                  tag="wbd2", name="wbd2")
            nc.gpsimd.memset(wbd2[:], 0.0)
            for g in range(2):
                for op in _weight_gen_ops(nc, g, wfin_all, wbd2, wbsb, se_sb):
                    op()

            # persistent image tiles; groups 0/1 primed here
            xts = []
            for i in range(n_groups):
                xt_i = xpool.tile([128, H, WP], BF16, tag=f"xt{i}",
                                  name=f"xt{i}")
                xts.append(xt_i)
            _load_rows(nc, xts[0], x_d, 0, first_small=True)
            _load_rows(nc, xts[1], x_d, 1)

            # --- PE warmup: 64x64 tiled matmuls ramp the p-state. The
            # (bank, partition-half) pair must not recur within the PE's
            # in-flight window — concurrent same-bank same-partition writes
            # from different tiles crash the device.
            for k in range(WARMUP_MM):
                i2, j2, bk = (k // 2) % 2, k % 2, k % 8
                nc.tensor.matmul(
                    ps[bk][64 * j2:64 * j2 + 64, :, :],
                    warm[64 * i2:64 * i2 + 64, 512:576],
                    warm[64 * i2:64 * i2 + 64, 0:512],
                    start=True, stop=True, skip_group_check=True,
                    tile_position=(64 * i2, 64 * j2),
                )

            from contextlib import nullcontext

            if reps > 1:
                assert reps % 4 == 0, "reps must be divisible by 4"
                rep_loop, n_bodies = tc.For_i(0, reps // 4, 1), 4
            else:
                rep_loop, n_bodies = nullcontext(), 1
            with rep_loop:
                for b in range(n_bodies):
                    _emit_body(nc, tc, x_d, y_d, wbsb, se_sb, bias_sb,
                               xts, wfin_all, wbd2, outp, ps,
                               reload_head=(reps > 1),
                               tail_warm=(warm if reps > 1 and
                                          b == n_bodies - 1 else None))

    if split_waits:
        _split_multiwait_insts(nc)
    return nc


def _weight_gen_ops(nc, g, wfin_all, wbd2, wbsb, se_sb):
    """wfin[(s,ch), g, t, o] = sum_n se[(s,ch),g,n] * wbsb[(s,ch),n,t,o] in
    bf16 (ACT scaled copy + 7 DVE fused multiply-adds), then 4 DVE copies of
    the per-sample blocks into the 2-sample block-diagonal stationary bank.
    One closure per instruction for staggered emission."""
    mult = mybir.AluOpType.mult
    add = mybir.AluOpType.add
    ident = mybir.ActivationFunctionType.Identity
    wfin = wfin_all[:, g]
    ops = [
        lambda: nc.scalar.activation(
            wfin, wbsb[:, 0], ident, scale=se_sb[:, g, 0:1]
        )
    ]
    for n in range(1, NUM):
        ops.append(
            lambda n=n: nc.vector.scalar_tensor_tensor(
                wfin, wbsb[:, n], se_sb[:, g, n : n + 1], wfin, mult, add
            )
        )
    for s in range(GROUP):
        o0 = O * (s % 2)
        ops.append(
            lambda s=s, o0=o0: nc.vector.tensor_copy(
                wbd2[32 * s:32 * s + 32, g, :, o0:o0 + O],
                wfin_all[32 * s:32 * s + 32, g],
            )
        )
    return ops


def _load_rows(nc, t, x_d, g, first_small=False):
    """x-image load as 8 row-chunk DMAs so the SDMA round-robin doesn't
    starve the store ring (packet-granularity queue switching)."""
    bounds = (0, 8, 24, 40, 56, 72, 88, 108, H) if first_small else (
        0, 16, 32, 48, 64, 80, 96, 112, H
    )
    for r0, r1 in zip(bounds, bounds[1:]):
        nc.sync.dma_start(out=t[:, r0:r1, :], in_=x_d[g, :, r0:r1, :])


def _emit_mm(nc, ps_b, j2, xt, wbd2g, i2, c, kh, kw, start, stop):
    t = kh * KK + kw  # canonical tap index in the weight bank
    # output rows 4c..4c+3 read x rows 4c+kh-1..4c+kh+2; restrict at edges
    or0 = 1 if (c == 0 and kh == 0) else 0
    or1 = 3 if (c == N_CHUNKS - 1 and kh == 2) else 4
    nc.tensor.matmul(
        ps_b[64 * j2:64 * j2 + 64, or0:or1, :],
        wbd2g[64 * i2:64 * i2 + 64, t, :],
        xt[64 * i2:64 * i2 + 64,
           4 * c + kh - 1 + or0:4 * c + kh - 1 + or1, kw:kw + W],
        start=start, stop=stop, skip_group_check=True,
        tile_position=(64 * i2, 64 * j2),
    )


def _emit_body(nc, tc, x_d, y_d, wbsb, se_sb, bias_sb, xts, wfin_all, wbd2,
               outp, ps, reload_head=False, tail_warm=None):
    ident = mybir.ActivationFunctionType.Identity
    n_groups = N_GROUPS

    # weight-gen rotation: g's weights generated two groups ahead; x images
    # for g+2 loaded during g's conv (and refreshed for the next body)
    gen_during = {0: (2,), 1: (3,), 2: (0,), 3: (1,)}
    load_during = {0: 2, 1: 3}
    if reload_head:
        load_during.update({2: 0, 3: 1})

    for g in range(n_groups):
        lg = load_during.get(g)
        if lg is not None:
            _load_rows(nc, xts[lg], x_d, lg)
        pending = []
        for gg in gen_during.get(g, ()):
            if reload_head or gg > g:
                pending += _weight_gen_ops(nc, gg, wfin_all, wbd2, wbsb, se_sb)
        xt = xts[g]
        wbd2g = wbd2[:, g]
        for p in range(N_PASSES):
            ph = 4 * (p % 2)
            # conv: 4 chunks x 4 samples, 9 taps; I fastest so consecutive
            # MMs never share a row group or a tile
            for ti, (kh, kw) in enumerate(TAPS):
                for q in range(2):
                    for j2 in range(2):
                        for i2 in range(2):
                            c = 4 * p + 2 * j2 + q
                            _emit_mm(nc, ps[2 * q + i2 + ph], j2, xt, wbd2g,
                                     i2, c, kh, kw,
                                     start=(ti == 0), stop=(ti == NTAPS - 1))
            # drains: bank 2q+i2+ph -> staging[i2][q]; ACT i2=0, DVE i2=1
            for i2 in range(2):
                ob = outp.tile([128, 2, ROWS_PER_CHUNK, W], BF16, name="ob")
                for q in range(2):
                    bk = 2 * q + i2 + ph
                    if i2 == 0:
                        nc.scalar.activation(ob[:, q], ps[bk][:], ident,
                                             bias=bias_sb[:, 0:1])
                    else:
                        nc.vector.tensor_scalar_add(ob[:, q], ps[bk][:],
                                                    bias_sb[:, 0:1])
                nc.scalar.dma_start(out=y_d[g, p, i2], in_=ob[:])
            # stagger weight-gen ops for the next groups between passes
            for op in pending[:2]:
                op()
            del pending[:2]
        for op in pending:
            op()

    if tail_warm is not None:
        # keep the PE clock ramped through the loop barrier's flush window
        for k in range(12):
            i2, j2, bk = (k // 2) % 2, k % 2, k % 8
            nc.tensor.matmul(
                ps[bk][64 * j2:64 * j2 + 64, :, :],
                tail_warm[64 * i2:64 * i2 + 64, 512:576],
                tail_warm[64 * i2:64 * i2 + 64, 0:512],
                start=True, stop=True, skip_group_check=True,
                tile_position=(64 * i2, 64 * j2),
            )


def _host_prep(inputs, inputs_se, weight, bias):
    """Shard + relayout the inputs for the 8 per-core programs."""
    inputs = np.asarray(inputs, dtype=np.float32)
    inputs_se = np.asarray(inputs_se, dtype=np.float32)
    weight = np.asarray(weight, dtype=np.float32)
    bias = np.asarray(bias, dtype=np.float32)

    # column-padded bf16 images: xpad[b, i, :, 1:129] = x[b, i]
    xpad = np.zeros((B, C, H, WP), dtype=BF16_NP)
    xpad[:, :, :, 1 : W + 1] = inputs.astype(BF16_NP)

    # wb[32s+i, n, t, o] = weight[o*288 + i*9 + t, n], replicated 4x
    wb = weight.reshape(O, C, NTAPS, NUM).transpose(1, 3, 2, 0)
    wbh = np.tile(
        np.ascontiguousarray(wb).astype(BF16_NP), (GROUP, 1, 1, 1)
    )  # [128, 8, 9, 32]
    bias_rep = np.ascontiguousarray(
        np.tile(bias, GROUP)[:, None].astype(np.float32)
    )  # [128, 1]

    in_maps = []
    for core in range(N_CORES):
        b0 = core * B_LOCAL
        xh = xpad[b0 : b0 + B_LOCAL].reshape(N_GROUPS, 128, H, WP)
        se_loc = inputs_se[b0 : b0 + B_LOCAL]           # [16, 8]
        # se[32*s+i, g, n] = se_loc[g*4+s, n]
        se_exp = np.repeat(
            se_loc.reshape(N_GROUPS, GROUP, NUM).transpose(1, 0, 2), 32, axis=0
        )
        in_maps.append(
            {
                "x": np.ascontiguousarray(xh),
                "wb": wbh,
                "se": np.ascontiguousarray(se_exp, dtype=np.float32),
                "bias": bias_rep,
            }
        )
    return in_maps


def _host_unpack(per_core_y):
    """[n_cores][4g, 8p, 2ii, 128(jj,k,o), 2q, 4r, 128w] bf16 -> [B,O,H,W]."""
    y = np.stack(per_core_y)
    y = y.reshape(N_CORES, N_GROUPS, N_PASSES, 2, 2, 2, O, 2,
                  ROWS_PER_CHUNK, W)
    # axes: core,g,p,ii,jj,k,o,q,r,w ; sample = 4g+2ii+k ;
    # H = 4*(4p + 2jj + q) + r
    y = y.transpose(0, 1, 3, 5, 6, 2, 4, 7, 8, 9)  # core,g,ii,k,o,p,jj,q,r,w
    return np.ascontiguousarray(
        y.reshape(B, O, H, W).astype(np.float32))


_NC_CACHE = {}


def kernel(inputs, inputs_se, weight, bias):
    if "nc" not in _NC_CACHE:
        _NC_CACHE["nc"] = build_program()
    nc = _NC_CACHE["nc"]
    in_maps = _host_prep(inputs, inputs_se, weight, bias)
    res = run_bass_kernel_spmd(nc, in_maps, list(range(N_CORES)))
    return _host_unpack([res.results[i]["y"] for i in range(N_CORES)])


# revision 3
# speedup vs baseline: 1.2034x; 1.0002x over previous
"""Per-sample dynamic 3x3 conv (DCConv2d) on 8 Trainium2 NeuronCores — v3.

PE runs in 64x64 tiled mode (4 tiles): row half I = 2 samples' channels
block-diagonal, col half J = output chunk. Each N=512 matmul covers 2
samples x 1 chunk x 1 tap, and the 4 tiles stream concurrently, so the
per-instruction dispatch cost (~34-60ns) is amortized over 2 chunk-units —
measured ~2x the work-rate of the v1 full-array block-diagonal layout.
Output is stored bf16 (host converts to f32); x is host-column-padded
only, row edges handled by row-restricted matmuls.

Structure per core (16 samples = 4 groups of 4):
  x     [4, 128, 128, 130] bf16    (group, 32s+ch, row, col+1)
  wb    [128, 8, 9, 32]    bf16    replicated expert bank
  se    [128, 4, 8]        f32     replicated se coefficients
  bias  [128, 1]           f32     bias replicated on 32-partition blocks
  y     [4, 8, 2, 128, 2, 4, 128] bf16  (g, pass, ii, (jj,k,o), q, r, w)

Per group: 8 passes of 4 chunks x 4 samples. Tile (I=64*ii, J=64*jj)
accumulates chunks (4p+2*jj+q) for q=0,1 over 9 taps into PSUM bank
2q+ii+4*(p%2) partitions J..J+63 (4 banks per pass, ping-pong with the
other 4). Weights: per-group per-sample [9,32] bank (ACT+7 DVE gen) then
4 DVE copies into a persistent 2-sample block-diagonal [128,4g,9,64] bank.
Drains split ACT/DVE with fused bias + bf16 cast; stores on the scalar
HWDGE ring; x loads on the sync ring.
"""

import numpy as np
import ml_dtypes

import concourse.bass as bass
import concourse.mybir as mybir
import concourse.tile as tile
from concourse.bass_utils import run_bass_kernel_spmd

N_CORES = 8
B, C, H, W = 128, 32, 128, 128
O = 32
NUM = 8
KK = 3
B_LOCAL = B // N_CORES          # 16
GROUP = 4                       # samples per group
N_GROUPS = B_LOCAL // GROUP     # 4
WP = W + 2                      # column-padded row width
ROWS_PER_CHUNK = 4
N_CHUNKS = H // ROWS_PER_CHUNK  # 32
N_PASSES = N_CHUNKS // 4        # 8 passes/group, 4 chunks/pass
NTAPS = KK * KK
WARMUP_MM = 72

F32 = mybir.dt.float32
BF16 = mybir.dt.bfloat16
BF16_NP = ml_dtypes.bfloat16

# full-coverage tap (kh=1,kw=1) first so start=True writes every element
TAPS = [(1, 1), (0, 0), (0, 1), (0, 2), (1, 0), (1, 2), (2, 0), (2, 1), (2, 2)]


def _split_multiwait_insts(nc):
    """This walrus build encodes at most one sync-wait per instruction; Tile's
    tail drain carries one wait per hardware proc used. Split the extras into
    single-wait NOPs on the same engine, inserted just before."""
    for f in nc.m.functions:
        for blk in f.blocks:
            new_list = []
            changed = False
            for inst in blk.instructions:
                si = inst.sync_info
                if si is not None and len(si.on_wait) > 1:
                    waits = list(si.on_wait)
                    for j, w in enumerate(waits[:-1]):
                        new_list.append(
                            mybir.InstNoOp(
                                name=f"{inst.name}-ws-{j}",
                                engine=inst.engine,
                                ins=[],
                                outs=[],
                                sync_info=mybir.SyncInfo(on_wait=[w], on_update=[]),
                            )
                        )
                    inst.sync_info = mybir.SyncInfo(
                        on_wait=[waits[-1]], on_update=list(si.on_update)
                    )
                    changed = True
                new_list.append(inst)
            if changed:
                blk.instructions = new_list


def build_program(split_waits=True, reps=1):
    n_groups = N_GROUPS
    nc = bass.Bass(
        "TRN2",
        target_bir_lowering=False,
        debug=False,
        num_devices=N_CORES,
        enable_partition_id=False,
    )
    x_d = nc.dram_tensor("x", [n_groups, 128, H, WP], BF16,
                         kind="ExternalInput").ap()
    wb_d = nc.dram_tensor("wb", [128, NUM, NTAPS, O], BF16,
                          kind="ExternalInput").ap()
    se_d = nc.dram_tensor("se", [128, n_groups, NUM], F32,
                          kind="ExternalInput").ap()
    bias_d = nc.dram_tensor("bias", [128, 1], F32, kind="ExternalInput").ap()
    y_d = nc.dram_tensor(
        "y", [n_groups, N_PASSES, 2, 128, 2, ROWS_PER_CHUNK, W], BF16,
        kind="ExternalOutput"
    ).ap()

    with tile.TileContext(nc) as tc:
        with (
            tc.tile_pool(name="xpool", bufs=1) as xpool,
            tc.tile_pool(name="wbpool", bufs=1) as wbpool,
            tc.tile_pool(name="wfin", bufs=1) as wfin_pool,
            tc.tile_pool(name="sepool", bufs=1) as sepool,
            tc.tile_pool(name="warm", bufs=1) as warmpool,
            tc.tile_pool(name="outp", bufs=8) as outp,
            tc.tile_pool(name="psum", bufs=1, space="PSUM") as psump,
        ):
            # --- ACT table preload + one-time loads ------------------------
            scratch = warmpool.tile([128, 1], F32, tag="scratch")
            nc.gpsimd.memset(scratch[:], 0.0)
            nc.scalar.activation(
                scratch[:], scratch[:], mybir.ActivationFunctionType.Identity
            )
            wbsb = wbpool.tile([128, NUM, NTAPS, O], BF16)
            nc.sync.dma_start(out=wbsb[:], in_=wb_d[:])
            se_sb = sepool.tile([128, n_groups, NUM], F32)
            nc.scalar.dma_start(out=se_sb[:], in_=se_d[:])
            bias_sb = sepool.tile([128, 1], F32)
            nc.scalar.dma_start(out=bias_sb[:], in_=bias_d[:])
            warm = warmpool.tile([128, 576], BF16, tag="warm")
            nc.gpsimd.memset(warm[:], 0.0)

            # 8 persistent PSUM bank tiles, viewed [128, 4 rows, 128 cols]
            ps = [psump.tile([128, ROWS_PER_CHUNK, W], F32, tag=f"ps{b}",
                             name=f"ps{b}") for b in range(8)]

            # per-sample weights [128,(g),9,32] and the persistent 2-sample
            # block-diagonal stationary bank [128,(g),9,64] (off-diagonal
            # zeros persist; only diagonal blocks are rewritten per group)
            wfin_all = wfin_pool.tile([128, n_groups, NTAPS, O], BF16,
                                      tag="wfin", name="wfin_all")
            wbd2 = wfin_pool.tile([128, n_groups, NTAPS, 2 * O], BF16,
                                  tag="wbd2", name="wbd2")
            nc.gpsimd.memset(wbd2[:], 0.0)
            for g in range(2):
                for op in _weight_gen_ops(nc, g, wfin_all, wbd2, wbsb, se_sb):
                    op()

            # persistent image tiles; groups 0/1 primed here
            xts = []
            for i in range(n_groups):
                xt_i = xpool.tile([128, H, WP], BF16, tag=f"xt{i}",
                                  name=f"xt{i}")
                xts.append(xt_i)
            _load_rows(nc, xts[0], x_d, 0, first_small=True)
            _load_rows(nc, xts[1], x_d, 1)

            # --- PE warmup: 64x64 tiled matmuls ramp the p-state. The
            # (bank, partition-half) pair must not recur within the PE's
            # in-flight window — concurrent same-bank same-partition writes
            # from different tiles crash the device.
            for k in range(WARMUP_MM):
                i2, j2, bk = (k // 2) % 2, k % 2, k % 8
                nc.tensor.matmul(
                    ps[bk][64 * j2:64 * j2 + 64, :, :],
                    warm[64 * i2:64 * i2 + 64, 512:576],
                    warm[64 * i2:64 * i2 + 64, 0:512],
                    start=True, stop=True, skip_group_check=True,
                    tile_position=(64 * i2, 64 * j2),
                )

            from contextlib import nullcontext

            if reps > 1:
                assert reps % 4 == 0, "reps must be divisible by 4"
                rep_loop, n_bodies = tc.For_i(0, reps // 4, 1), 4
            else:
                rep_loop, n_bodies = nullcontext(), 1
            with rep_loop:
                for b in range(n_bodies):
                    _emit_body(nc, tc, x_d, y_d, wbsb, se_sb, bias_sb,
                               xts, wfin_all, wbd2, outp, ps,
                               reload_head=(reps > 1),
                               tail_warm=(warm if reps > 1 and
                                          b == n_bodies - 1 else None))

    if split_waits:
        _split_multiwait_insts(nc)
    return nc


def _weight_gen_ops(nc, g, wfin_all, wbd2, wbsb, se_sb):
    """wfin[(s,ch), g, t, o] = sum_n se[(s,ch),g,n] * wbsb[(s,ch),n,t,o] in
    bf16 (ACT scaled copy + 7 DVE fused multiply-adds), then 4 DVE copies of
    the per-sample blocks into the 2-sample block-diagonal stationary bank.
    One closure per instruction for staggered emission."""
    mult = mybir.AluOpType.mult
    add = mybir.AluOpType.add
    ident = mybir.ActivationFunctionType.Identity
    wfin = wfin_all[:, g]
    ops = [
        lambda: nc.scalar.activation(
            wfin, wbsb[:, 0], ident, scale=se_sb[:, g, 0:1]
        )
    ]
    for n in range(1, NUM):
        ops.append(
            lambda n=n: nc.vector.scalar_tensor_tensor(
                wfin, wbsb[:, n], se_sb[:, g, n : n + 1], wfin, mult, add
            )
        )
    for s in range(GROUP):
        o0 = O * (s % 2)
        ops.append(
            lambda s=s, o0=o0: nc.vector.tensor_copy(
                wbd2[32 * s:32 * s + 32, g, :, o0:o0 + O],
                wfin_all[32 * s:32 * s + 32, g],
            )
        )
    return ops


def _load_rows(nc, t, x_d, g, first_small=False):
    """x-image load as 8 row-chunk DMAs so the SDMA round-robin doesn't
    starve the store ring (packet-granularity queue switching)."""
    bounds = (0, 8, 24, 40, 56, 72, 88, 108, H) if first_small else (
        0, 16, 32, 48, 64, 80, 96, 112, H
    )
    for r0, r1 in zip(bounds, bounds[1:]):
        nc.sync.dma_start(out=t[:, r0:r1, :], in_=x_d[g, :, r0:r1, :])


def _emit_mm(nc, ps_b, j2, xt, wbd2g, i2, c, kh, kw, start, stop):
    t = kh * KK + kw  # canonical tap index in the weight bank
    # output rows 4c..4c+3 read x rows 4c+kh-1..4c+kh+2; restrict at edges
    or0 = 1 if (c == 0 and kh == 0) else 0
    or1 = 3 if (c == N_CHUNKS - 1 and kh == 2) else 4
    nc.tensor.matmul(
        ps_b[64 * j2:64 * j2 + 64, or0:or1, :],
        wbd2g[64 * i2:64 * i2 + 64, t, :],
        xt[64 * i2:64 * i2 + 64,
           4 * c + kh - 1 + or0:4 * c + kh - 1 + or1, kw:kw + W],
        start=start, stop=stop, skip_group_check=True,
        tile_position=(64 * i2, 64 * j2),
    )


def _emit_body(nc, tc, x_d, y_d, wbsb, se_sb, bias_sb, xts, wfin_all, wbd2,
               outp, ps, reload_head=False, tail_warm=None):
    ident = mybir.ActivationFunctionType.Identity
    n_groups = N_GROUPS

    # weight-gen rotation: g's weights generated two groups ahead; x images
    # for g+2 loaded during g's conv (and refreshed for the next body)
    gen_during = {0: (2,), 1: (3,), 2: (0,), 3: (1,)}
    load_during = {0: 2, 1: 3}
    if reload_head:
        load_during.update({2: 0, 3: 1})

    for g in range(n_groups):
        lg = load_during.get(g)
        pending = []
        for gg in gen_during.get(g, ()):
            if reload_head or gg > g:
                pending += _weight_gen_ops(nc, gg, wfin_all, wbd2, wbsb, se_sb)
        xt = xts[g]
        wbd2g = wbd2[:, g]
        for p in range(N_PASSES):
            ph = 4 * (p % 2)
            if lg is not None:
                # spread the next group's x load: one 16-row chunk per pass
                nc.sync.dma_start(out=xts[lg][:, 16 * p:16 * p + 16, :],
                                  in_=x_d[lg, :, 16 * p:16 * p + 16, :])
            # conv: 4 chunks x 4 samples, 9 taps; I fastest so consecutive
            # MMs never share a row group or a tile
            for ti, (kh, kw) in enumerate(TAPS):
                for q in range(2):
                    for j2 in range(2):
                        for i2 in range(2):
                            c = 4 * p + 2 * j2 + q
                            _emit_mm(nc, ps[2 * q + i2 + ph], j2, xt, wbd2g,
                                     i2, c, kh, kw,
                                     start=(ti == 0), stop=(ti == NTAPS - 1))
            # drains: bank 2q+i2+ph -> staging[i2][q]; ACT i2=0, DVE i2=1
            for i2 in range(2):
                ob = outp.tile([128, 2, ROWS_PER_CHUNK, W], BF16, name="ob")
                for q in range(2):
                    bk = 2 * q + i2 + ph
                    if i2 == 0:
                        nc.scalar.activation(ob[:, q], ps[bk][:], ident,
                                             bias=bias_sb[:, 0:1])
                    else:
                        nc.vector.tensor_scalar_add(ob[:, q], ps[bk][:],
                                                    bias_sb[:, 0:1])
                nc.scalar.dma_start(out=y_d[g, p, i2], in_=ob[:])
            # stagger weight-gen ops for the next groups between passes
            for op in pending[:2]:
                op()
            del pending[:2]
        for op in pending:
            op()

    if tail_warm is not None:
        # keep the PE clock ramped through the loop barrier's flush window
        for k in range(12):
            i2, j2, bk = (k // 2) % 2, k % 2, k % 8
            nc.tensor.matmul(
                ps[bk][64 * j2:64 * j2 + 64, :, :],
                tail_warm[64 * i2:64 * i2 + 64, 512:576],
                tail_warm[64 * i2:64 * i2 + 64, 0:512],
                start=True, stop=True, skip_group_check=True,
                tile_position=(64 * i2, 64 * j2),
            )


def _host_prep(inputs, inputs_se, weight, bias):
    """Shard + relayout the inputs for the 8 per-core programs."""
    inputs = np.asarray(inputs, dtype=np.float32)
    inputs_se = np.asarray(inputs_se, dtype=np.float32)
    weight = np.asarray(weight, dtype=np.float32)
    bias = np.asarray(bias, dtype=np.float32)

    # column-padded bf16 images: xpad[b, i, :, 1:129] = x[b, i]
    xpad = np.zeros((B, C, H, WP), dtype=BF16_NP)
    xpad[:, :, :, 1 : W + 1] = inputs.astype(BF16_NP)

    # wb[32s+i, n, t, o] = weight[o*288 + i*9 + t, n], replicated 4x
    wb = weight.reshape(O, C, NTAPS, NUM).transpose(1, 3, 2, 0)
    wbh = np.tile(
        np.ascontiguousarray(wb).astype(BF16_NP), (GROUP, 1, 1, 1)
    )  # [128, 8, 9, 32]
    bias_rep = np.ascontiguousarray(
        np.tile(bias, GROUP)[:, None].astype(np.float32)
    )  # [128, 1]

    in_maps = []
    for core in range(N_CORES):
        b0 = core * B_LOCAL
        xh = xpad[b0 : b0 + B_LOCAL].reshape(N_GROUPS, 128, H, WP)
        se_loc = inputs_se[b0 : b0 + B_LOCAL]           # [16, 8]
        # se[32*s+i, g, n] = se_loc[g*4+s, n]
        se_exp = np.repeat(
            se_loc.reshape(N_GROUPS, GROUP, NUM).transpose(1, 0, 2), 32, axis=0
        )
        in_maps.append(
            {
                "x": np.ascontiguousarray(xh),
                "wb": wbh,
                "se": np.ascontiguousarray(se_exp, dtype=np.float32),
                "bias": bias_rep,
            }
        )
    return in_maps


def _host_unpack(per_core_y):
    """[n_cores][4g, 8p, 2ii, 128(jj,k,o), 2q, 4r, 128w] bf16 -> [B,O,H,W]."""
    y = np.stack(per_core_y)
    y = y.reshape(N_CORES, N_GROUPS, N_PASSES, 2, 2, 2, O, 2,
                  ROWS_PER_CHUNK, W)
    # axes: core,g,p,ii,jj,k,o,q,r,w ; sample = 4g+2ii+k ;
    # H = 4*(4p + 2jj + q) + r
    y = y.transpose(0, 1, 3, 5, 6, 2, 4, 7, 8, 9)  # core,g,ii,k,o,p,jj,q,r,w
    return np.ascontiguousarray(
        y.reshape(B, O, H, W).astype(np.float32))


_NC_CACHE = {}


def kernel(inputs, inputs_se, weight, bias):
    if "nc" not in _NC_CACHE:
        _NC_CACHE["nc"] = build_program()
    nc = _NC_CACHE["nc"]
    in_maps = _host_prep(inputs, inputs_se, weight, bias)
    res = run_bass_kernel_spmd(nc, in_maps, list(range(N_CORES)))
    return _host_unpack([res.results[i]["y"] for i in range(N_CORES)])
